# revision 19
# baseline (speedup 1.0000x reference)
"""Trainium2 Bass kernel for nn_DIAGCN (RGCN + GraphConv + classifier over
block-diagonal dialog graphs), SPMD over 8 NeuronCores.

Strategy (v2 — fp16 streaming, window-2 fused into the S-matmul)
----------------------------------------------------------------
The dialog graph is a causal 5-tap window (edges i -> i+o, o = 0..4, within
each 100-utterance dialog), and relation_type(i,j) = spk[i]*spk[j] with spk
derived from self-edges.  Every per-node linear map commutes with both the
window sum W(.) (row-mixing) and per-node diagonal scalings, so the whole
network folds into 7-wide channels:

    out = W(g0) + f0
    g0  = sum of A-channels of V,  f0 = sum of B-channels of V
    V   = coef .* W(spkmask .* (x @ Wbig))  (+ plain RA/FSC/mask channels)

Device pipeline per 504-column tile:
  1 fp16 DMA (1 MB) -> 8 fp16 matmuls (ps = Wbig^T x, fp32 PSUM)
  -> DVE evict (ps * spk-mask, fp16) -> 3 DVE shifted adds (5-tap window)
  -> GpSimd coef multiply -> 6 small matmuls: po = S_B^T V + sum_o S_A^T V
  shifted by o columns (the second 5-tap window W(g0) runs on the tensor
  engine as shifted-AP PSUM accumulation) -> ACT evict -> y DMA.

Layout: nodes sharded by dialog (no cross-core edges), 63 dialogs per core,
each dialog = 4 zero gap columns + 100 data columns so window sums never
leak across dialogs.  Z/T1/V carry a 4-column zero halo so shifted reads
never go out of bounds (no edge special-casing).  x ships transposed,
fp16, k-major ([tile][partition][k-block][col]).
"""
import ml_dtypes
import numpy as np

# ---------------------------------------------------------------- constants
B, L, FUT = 500, 100, 4
N = B * L
IN, HID, NCLS = 1024, 512, 7
NCORES = 8
GAP = 4
DLG = L + GAP            # 104 columns per dialog
DPC = 63                 # padded dialogs per core
COLS = DPC * DLG         # 6552 columns per core
NT = 13                  # column tiles
NTC = COLS // NT         # 504
KB = IN // 128           # 8 contraction blocks
M = 80                   # Wbig columns (psum partitions)
M2 = 14                  # S-matmul columns (7 A-sum + 7 B-sum)
HALO = 4

D_COUNTS = [63] * 7 + [59]
D_STARTS = np.concatenate([[0], np.cumsum(D_COUNTS)])[:-1]

# Wbig column map == V row map
R_A0S, R_A1S, R_B0S, R_B1S = 0, 7, 14, 21
R_A0, R_B0 = 32, 39
R_NV = 46                # Z row: real-mask, windowed -> neighbor-count channel
R_ZERO = 47
R_RA, R_FSC = 64, 71
R_MASKP = 78             # V row: real-mask, plain (bias-constant channel)
WIN_ROWS = 48


def _data_cols():
    d = np.arange(DPC)[:, None]
    u = np.arange(L)[None, :]
    return d * DLG + GAP + u  # [DPC, L]


# ---------------------------------------------------------------- host prep
def _check_graph(edges, relation_type):
    i = np.arange(L)[:, None]
    off = np.arange(FUT + 1)[None, :]
    tl = i + off
    valid = tl < L
    sl = np.broadcast_to(i, tl.shape)[valid]
    tl = tl[valid]
    base = (np.arange(B) * L)[:, None]
    src = (base + sl[None, :]).reshape(-1)
    tgt = (base + tl[None, :]).reshape(-1)
    if edges.shape != (2, src.size) or not (
        np.array_equal(edges[0], src) and np.array_equal(edges[1], tgt)
    ):
        raise ValueError("edge structure does not match the DIAGCN pattern")
    sel = edges[0] == edges[1]
    spk = np.zeros(N, dtype=np.float64)
    spk[edges[0][sel]] = relation_type[sel]
    return spk


def _host_prep(x, edges, relation_type, w_rel, w_root, b_rgcn,
               w_gc_rel, w_gc_root, b_gc, w_skip, b_skip, w_clf, b_clf):
    x = np.asarray(x, dtype=np.float32)
    edges = np.asarray(edges)
    relation_type = np.asarray(relation_type)
    spk = _check_graph(edges, relation_type)

    tgt = edges[1]
    c1 = np.bincount(tgt[relation_type == 1], minlength=N).astype(np.float64)
    c0 = np.bincount(tgt[relation_type == 0], minlength=N).astype(np.float64)
    ic0 = 1.0 / np.maximum(c0, 1.0)
    ic1 = 1.0 / np.maximum(c1, 1.0)
    ic0s = ic0 * spk
    ic1s = ic1 * spk

    f8 = lambda a: np.asarray(a, dtype=np.float64)
    w_rel, w_root, w_gc_rel, w_gc_root, w_skip, w_clf = map(
        f8, (w_rel, w_root, w_gc_rel, w_gc_root, w_skip, w_clf))
    b_rgcn, b_gc, b_skip, b_clf = map(f8, (b_rgcn, b_gc, b_skip, b_clf))

    wA = w_gc_rel @ w_clf
    wB = w_gc_root @ w_clf
    Wbig = np.zeros((IN, M), dtype=np.float64)
    Wbig[:, R_A0S:R_A0S + 7] = w_rel[0] @ wA
    Wbig[:, R_A1S:R_A1S + 7] = w_rel[1] @ wA
    Wbig[:, R_B0S:R_B0S + 7] = w_rel[0] @ wB
    Wbig[:, R_B1S:R_B1S + 7] = w_rel[1] @ wB
    Wbig[:, R_A0:R_A0 + 7] = w_rel[0] @ wA
    Wbig[:, R_B0:R_B0 + 7] = w_rel[0] @ wB
    Wbig[:, R_RA:R_RA + 7] = w_root @ wA
    Wbig[:, R_FSC:R_FSC + 7] = w_root @ wB + w_skip @ w_clf
    # [128 partitions, KB, M]: partition p holds weight rows {k*128+p}
    Wbig = np.ascontiguousarray(
        Wbig.astype(np.float16).reshape(KB, 128, M).swapaxes(0, 1))

    cA = b_rgcn @ wA
    cBc = b_rgcn @ wB + (b_gc + b_skip) @ w_clf + b_clf
    # cols 0:7 = S0 (o=0 term: A-sum + B-sum + both bias channels);
    # cols 7:14 = S_sh (o=1..4 shifted terms: A-sum only, no bias channels)
    S = np.zeros((M, M2), dtype=np.float64)
    for i in range(7):
        for cset in (i, 7 + i):
            S[R_A0S + i, cset] = 1.0
            S[R_A1S + i, cset] = 1.0
            S[R_A0 + i, cset] = 1.0
            S[R_RA + i, cset] = 1.0
        S[R_B0S + i, i] = 1.0
        S[R_B1S + i, i] = 1.0
        S[R_B0 + i, i] = 1.0
        S[R_FSC + i, i] = 1.0
        S[R_NV, i] = cA[i]
        S[R_MASKP, i] = cBc[i]
    S = S.astype(np.float16)

    dc = _data_cols()
    mask_col = np.zeros(COLS, dtype=np.float16)
    mask_col[dc.reshape(-1)] = 1.0
    # Z rows 46 (mask) and 47 (zero), with zero halo
    zrow = np.zeros((2, HALO + COLS), dtype=np.float16)
    zrow[0, HALO:] = mask_col
    # V row 78 (mask), with zero halo
    mrow = np.zeros((1, HALO + COLS), dtype=np.float16)
    mrow[0, HALO:] = mask_col

    in_maps = []
    unshard_info = []
    for c in range(NCORES):
        nd = D_COUNTS[c]
        g0 = D_STARTS[c]
        cols_real = dc[:nd].reshape(-1)
        nodes_real = g0 * L + np.arange(nd * L)

        xt = np.zeros((IN, COLS), dtype=np.float32)
        xt[:, cols_real] = x[nodes_real].T
        # [NT, 128, KB, NTC] fp8 e3m4: per-tile DMA is [128 partitions, 4 KiB]
        # (x ~ N(0,1) fits e3m4's +-15.5 range; the folded weights stay fp16
        # and the PE accepts mixed fp16 x fp8e3 operands)
        xts = np.ascontiguousarray(
            xt.reshape(KB, 128, NT, NTC).transpose(2, 1, 0, 3)
        ).astype(ml_dtypes.float8_e3m4)

        def vec_to_cols(v):
            out = np.zeros(COLS, dtype=np.float64)
            out[cols_real] = v[nodes_real]
            return out

        spk_c = vec_to_cols(spk)
        ic0_c = vec_to_cols(ic0)
        ic0s_c = vec_to_cols(ic0s)
        ic1s_c = vec_to_cols(ic1s)

        spkm = np.ones((46, COLS), dtype=np.float16)
        spkm[0:28] = spk_c.astype(np.float16)
        coefr = np.zeros((WIN_ROWS, COLS), dtype=np.float16)
        coefr[R_A0S:R_A0S + 7] = -ic0s_c
        coefr[R_A1S:R_A1S + 7] = ic1s_c
        coefr[R_B0S:R_B0S + 7] = -ic0s_c
        coefr[R_B1S:R_B1S + 7] = ic1s_c
        coefr[R_A0:R_A0 + 7] = ic0_c
        coefr[R_B0:R_B0 + 7] = ic0_c
        coefr[R_NV] = mask_col

        in_maps.append(dict(
            xt=xts, wbig=Wbig, smat=S, spkm=spkm, coefr=coefr,
            zrow=zrow, mrow=mrow,
        ))
        unshard_info.append((nodes_real, cols_real))
    return in_maps, unshard_info


# ---------------------------------------------------------------- bass kernel
_COMPILED = None


def _build():
    import concourse.bass as bass
    from concourse import bacc
    import concourse.mybir as mybir
    from concourse.tile import TileContext

    f16 = mybir.dt.float16
    f8 = mybir.dt.float8e3
    f32 = mybir.dt.float32
    ADD = mybir.AluOpType.add
    MUL = mybir.AluOpType.mult

    nc = bacc.Bacc("TRN2", target_bir_lowering=False, debug=False,
                   num_devices=NCORES)
    xt_d = nc.dram_tensor("xt", [NT, 128, KB, NTC], f8, kind="ExternalInput")
    wbig_d = nc.dram_tensor("wbig", [128, KB, M], f16, kind="ExternalInput")
    smat_d = nc.dram_tensor("smat", [M, M2], f16, kind="ExternalInput")
    spkm_d = nc.dram_tensor("spkm", [46, COLS], f16, kind="ExternalInput")
    coefr_d = nc.dram_tensor("coefr", [WIN_ROWS, COLS], f16, kind="ExternalInput")
    zrow_d = nc.dram_tensor("zrow", [2, HALO + COLS], f16, kind="ExternalInput")
    mrow_d = nc.dram_tensor("mrow", [1, HALO + COLS], f16, kind="ExternalInput")
    y_d = nc.dram_tensor("y", [NT, NCLS, NTC], f16, kind="ExternalOutput")

    with TileContext(nc) as tc:
        with (
            tc.tile_pool(name="const", bufs=1) as cpool,
            tc.tile_pool(name="xin", bufs=NT) as xpool,
            tc.tile_pool(name="wrk", bufs=3) as wpool,
            tc.tile_pool(name="psum", bufs=4, space="PSUM") as ppool,
            tc.tile_pool(name="psum2", bufs=2, space="PSUM") as p2pool,
            tc.tile_pool(name="pwarm", bufs=1, space="PSUM") as pwarm,
        ):
            wsb = cpool.tile([128, KB, M], f16)
            nc.sync.dma_start(wsb[:], wbig_d[:])
            ssb = cpool.tile([M, M2], f16)
            nc.sync.dma_start(ssb[:], smat_d[:])

            tSP = cpool.tile([46, COLS], f16)          # spk/ones evict mask
            tCF = cpool.tile([WIN_ROWS, COLS], f16)    # window coefficients
            tZ = cpool.tile([WIN_ROWS, HALO + COLS], f16)   # raw window input
            tT1 = cpool.tile([WIN_ROWS, HALO + COLS], f16)  # window stage 1
            tV = cpool.tile([M, HALO + COLS], f16)          # S-matmul input

            # warm-up scratch memset runs FIRST so the PE can start ramping
            # while the big tV memset is still going
            scr = cpool.tile([128, NTC], f16)
            nc.vector.memset(scr[:], 0.0)
            # halo zero-fills are cheap DVE memsets (no DMA semaphores)
            nc.vector.memset(tV[:], 0.0)  # rows 46:64/79 are read by S-matmul
            nc.vector.memset(tZ[0:WIN_ROWS, 0:HALO], 0.0)
            nc.vector.memset(tT1[0:WIN_ROWS, 0:HALO], 0.0)
            nc.scalar.dma_start(tZ[R_NV:R_ZERO + 1, :], zrow_d[:])
            nc.scalar.dma_start(tV[R_MASKP:R_MASKP + 1, :], mrow_d[:])

            # spk/coef ship as per-tile column chunks on the ACT ring, two
            # tiles ahead of use — bulk [46, COLS] loads land almost entirely
            # on two SDMA engines and straggle ~20us behind the xt flood
            def spc_chunk(t):
                c0 = t * NTC
                nc.scalar.dma_start(tSP[:, c0:c0 + NTC], spkm_d[:, c0:c0 + NTC])
                nc.scalar.dma_start(tCF[:, c0:c0 + NTC], coefr_d[:, c0:c0 + NTC])

            spc_chunk(0)
            spc_chunk(1)

            # ~28 dummy matmuls on a memset scratch keep the PE busy through
            # the initial DMA window so HAM un-throttles (1.2 -> 2.4 GHz)
            # before the first real matmul arrives
            pw = pwarm.tile([M, NTC], f32)
            for w in range(28):
                nc.tensor.matmul(pw[:], scr[:, 0:M], scr[:],
                                 start=True, stop=True)

            LAG = 2  # S-matmuls trail the big matmuls so the PE never stalls

            def small_mms(t):
                c0 = t * NTC
                h0 = HALO + c0
                # po = S0^T V + sum_{o=1..4} S_sh^T V<<o  (second window on PE)
                po = p2pool.tile([NCLS, NTC], f32)
                nc.tensor.matmul(po[:], ssb[:, 0:7], tV[0:M, h0:h0 + NTC],
                                 start=True, stop=False)
                for o in range(1, FUT + 1):
                    nc.tensor.matmul(
                        po[:], ssb[:, 7:14], tV[0:M, h0 - o:h0 + NTC - o],
                        start=False, stop=(o == FUT))
                out_t = wpool.tile([NCLS, NTC], f16, tag="OUT")
                nc.scalar.copy(out_t[:], po[:])
                nc.scalar.dma_start(y_d[t], out_t[:])

            for t in range(NT):
                c0 = t * NTC
                h0 = HALO + c0
                xt_t = xpool.tile([128, KB, NTC], f8)
                nc.sync.dma_start(xt_t[:], xt_d[t])
                if t + 2 < NT:
                    spc_chunk(t + 2)
                ps = ppool.tile([M, NTC], f32)
                for k in range(KB):
                    nc.tensor.matmul(
                        ps[:], wsb[:, k, :], xt_t[:, k, :],
                        start=(k == 0), stop=(k == KB - 1))
                if t >= LAG:
                    small_mms(t - LAG)

                # evict: Z = ps * spk-mask (rows 0:28 spk, 32:46 ones)
                nc.vector.tensor_tensor(
                    tZ[0:46, h0:h0 + NTC], ps[0:46], tSP[:, c0:c0 + NTC], MUL)
                # plain channels straight to V
                nc.scalar.copy(tV[R_RA:R_RA + 14, h0:h0 + NTC],
                               ps[R_RA:R_RA + 14])

                # 5-tap causal window: t1 = z + sh1(z); t2 = t1 + sh2(t1);
                # wt = t2 + sh4(z)  (halos make shifted reads uniform)
                nc.vector.tensor_tensor(
                    tT1[:, h0:h0 + NTC], tZ[:, h0:h0 + NTC],
                    tZ[:, h0 - 1:h0 + NTC - 1], ADD)
                T2 = wpool.tile([WIN_ROWS, NTC], f16, tag="T2")
                nc.vector.tensor_tensor(
                    T2[:], tT1[:, h0:h0 + NTC], tT1[:, h0 - 2:h0 + NTC - 2], ADD)
                WT = wpool.tile([WIN_ROWS, NTC], f16, tag="WT")
                nc.vector.tensor_tensor(
                    WT[:], T2[:], tZ[:, h0 - 4:h0 + NTC - 4], ADD)
                # per-target coefficients (GpSimd to offload the DVE)
                nc.gpsimd.tensor_tensor(
                    tV[0:WIN_ROWS, h0:h0 + NTC], WT[:], tCF[:, c0:c0 + NTC], MUL)

            for t in range(NT - LAG, NT):
                small_mms(t)
    nc.compile()
    return nc


def _get_compiled():
    global _COMPILED
    if _COMPILED is None:
        _COMPILED = _build()
    return _COMPILED


def _run(in_maps, trace=False):
    from concourse.bass_utils import run_bass_kernel_spmd
    nc = _get_compiled()
    return run_bass_kernel_spmd(nc, in_maps, list(range(NCORES)), trace=trace)


def kernel(**inputs) -> np.ndarray:
    in_maps, unshard_info = _host_prep(**inputs)
    res = _run(in_maps)
    out = np.zeros((N, NCLS), dtype=np.float32)
    for c in range(NCORES):
        nodes_real, cols_real = unshard_info[c]
        y = res.results[c]["y"].transpose(1, 0, 2).reshape(NCLS, COLS)
        out[nodes_real] = y[:, cols_real].T.astype(np.float32)
    return out
